# revision 21
# baseline (speedup 1.0000x reference)
"""Additive attention (Bahdanau) Trainium2 Bass kernel.

Problem (hardcoded shapes):
  B=4, LQ=256, LK=512, DQ=DK=DV=512, H=256, f32.
  out[b,q,v] = sum_k softmax_k(score)[b,q,k] * values[b,k,v]
  score[b,q,k] = sum_h wv[h] * tanh( (queries@Wq)[b,q,h] + (keys@Wk)[b,k,h] )
  with softmax masked to k < valid_lens[b].

Sharding: 8 cores, each core handles ALL batches but a 32-row slice of LQ
(q-rows c*32:(c+1)*32 of every batch).  No collectives: host slices inputs
and concatenates outputs.

Device-side layout: h lives on SBUF partitions.
  qfT[h, q] and kfT[h, k] come straight out of PE matmuls (host pre-transposes
  queries and keys so the d-contraction is on partitions).  Per (q, h-chunk),
  DVE builds tanh-inputs kfT + qfT[:,q] in bf16 (per-partition scalar add,
  2x packed mode) -- a few per (b,hc) go through ACT's fused bias+tanh
  instead, to balance the two engines.  ACT does one big tanh per 16-q block,
  and PE reduces over h against wv with M=1 matmuls packed 4-to-a-PSUM-bank
  via 32-aligned col-tiling.  Softmax skips max-subtraction (|score| <=
  |wv|_1 ~ 13, exp can't overflow); the denominator comes free from the
  exp's accum_out.  Everything is trimmed to k < valid_lens[b] (host-known
  at build time), which skips the masked work entirely.
"""
import os
from contextlib import ExitStack

import numpy as np

import concourse.bacc as bacc
import concourse.bass as bass
import concourse.tile as tile
from concourse import mybir
from concourse import bass_utils

AFT = mybir.ActivationFunctionType
F32 = mybir.dt.float32
BF16 = mybir.dt.bfloat16

B, LQ, LK = 4, 256, 512
DQ, DK, DV = 512, 512, 512
H = 256
N_CORES = 8
QS = LQ // N_CORES  # 32 q rows per core
QBLK = 16           # q rows per tanh batch
N_BIAS = 3          # q rows per (b, hc, blk0) routed through ACT bias-tanh
N_GP = 0            # q rows per (b, hc, blk) routed through GPSIMD adds
HC = H // 128       # h chunks (2)
DC = DQ // 128      # d chunks (4)

_PROGRAM_CACHE: dict = {}
last_results = None  # for test harness introspection


def build_body(tc, outs, ins, vbs):
    """Emit the per-core program.  outs/ins are dicts of bass.AP (DRAM)."""
    nc = tc.nc
    ctx = ExitStack()
    with ctx:
        consts = ctx.enter_context(tc.tile_pool(name="consts", bufs=1))
        kin_p = ctx.enter_context(tc.tile_pool(name="kin", bufs=4))
        kf_p = ctx.enter_context(tc.tile_pool(name="kf", bufs=4))
        s_p = ctx.enter_context(tc.tile_pool(name="s", bufs=3))
        st_p = ctx.enter_context(tc.tile_pool(name="st", bufs=12))
        sm_p = ctx.enter_context(tc.tile_pool(name="sm", bufs=3))
        at_p = ctx.enter_context(tc.tile_pool(name="at", bufs=4))
        val_p = ctx.enter_context(tc.tile_pool(name="val", bufs=4))
        ob_p = ctx.enter_context(tc.tile_pool(name="ob", bufs=2))
        # PSUM pools: 4+2+2 = 8 banks exactly (projections and score tiles
        # share one 4-deep pool)
        main_ps = ctx.enter_context(tc.tile_pool(name="mainps", bufs=4, space="PSUM"))
        proj_ps = main_ps
        sc_ps = main_ps
        tr_ps = ctx.enter_context(tc.tile_pool(name="trps", bufs=2, space="PSUM"))
        out_ps = ctx.enter_context(tc.tile_pool(name="outps", bufs=2, space="PSUM"))

        # ---- constants (kfT inputs first: they gate the critical path) ----
        wk_t = consts.tile([128, DC, H], BF16)
        nc.scalar.dma_start(wk_t[:], ins["Wk"].rearrange("(c p) h -> p c h", p=128))
        wq_t = consts.tile([128, DC, H], BF16)
        nc.scalar.dma_start(wq_t[:], ins["Wq"].rearrange("(c p) h -> p c h", p=128))
        qt_t = consts.tile([128, DC, 128], BF16)
        nc.scalar.dma_start(qt_t[:], ins["qT"].rearrange("(c p) q -> p c q", p=128))
        wvc = consts.tile([128, HC], BF16)
        nc.scalar.dma_start(wvc[:], ins["wv_col"][:])
        wvp0 = consts.tile([128, 128], BF16)
        nc.scalar.dma_start(wvp0[:], ins["wv_pad0"][:])
        eye = consts.tile([32, 32], BF16)
        nc.scalar.dma_start(eye[:], ins["eye32"][:])

        # ---- all keysT DMAs upfront (no engine time); kfT projections are
        # software-pipelined: batch i+1's projection is emitted after batch
        # i's first score block so its DVE cast doesn't front-run the
        # critical adds.  qfT right after the first kfT. ----
        border = sorted(range(B), key=lambda b: -vbs[b])
        kts = {}
        for b in border:
            vb = vbs[b]
            vbe = vb + (vb & 1)
            kt = kin_p.tile([128, DC, 512], BF16, tag="kin")
            nc.sync.dma_start(
                kt[:, :, 0:vbe],
                ins["keysT"][b].rearrange("(c p) k -> p c k", p=128)[:, :, 0:vbe])
            kts[b] = kt
        kfs = {}

        def emit_kft(b):
            vb_ = vbs[b]
            vbe_ = vb_ + (vb_ & 1)
            kf_ = kf_p.tile([128, HC, 512], BF16, tag="kf")
            for hc_ in range(HC):
                kps = proj_ps.tile([128, 512], F32, tag="ps512")
                for dc_ in range(DC):
                    nc.tensor.matmul(
                        kps[:, 0:vbe_],
                        wk_t[:, dc_, hc_ * 128:(hc_ + 1) * 128],
                        kts[b][:, dc_, 0:vbe_],
                        start=(dc_ == 0), stop=(dc_ == DC - 1),
                    )
                nc.vector.tensor_copy(kf_[:, hc_, 0:vbe_], kps[:, 0:vbe_])
            kfs[b] = kf_

        emit_kft(border[0])
        # ---- qfT[h, 4b*32q] ----
        qf = consts.tile([128, HC, 128], F32)
        for hc in range(HC):
            qps = proj_ps.tile([128, 512], F32, tag="ps512")
            for dc in range(DC):
                nc.tensor.matmul(
                    qps[:, 0:128],
                    wq_t[:, dc, hc * 128:(hc + 1) * 128],
                    qt_t[:, dc, :],
                    start=(dc == 0), stop=(dc == DC - 1),
                )
            nc.vector.tensor_copy(qf[:, hc, :], qps[:, 0:128])

        # ---- values stream (bf16) ----
        all_vts = {}
        for b in border:
            vb = vbs[b]
            nkc_b = (vb + 127) // 128
            nfull = vb // 128
            vt = val_p.tile([128, nkc_b, DV], BF16, tag=f"val{nkc_b}")
            if nfull:
                nc.sync.dma_start(
                    vt[:, 0:nfull, :],
                    ins["values"][b, 0:nfull * 128, :].rearrange(
                        "(c p) v -> p c v", p=128))
            if vb % 128:
                nc.sync.dma_start(
                    vt[0:vb % 128, nfull, :],
                    ins["values"][b, nfull * 128:vb, :])
            all_vts[b] = [(vt[:, kc, :], min(128, vb - kc * 128))
                          for kc in range(nkc_b)]

        for bi, b in enumerate(border):
            vb = vbs[b]
            vbe = vb + (vb & 1)  # even pad so DVE tensor_scalar packs 2/cycle
            nkc = (vb + 127) // 128
            kf = kfs[b]

            # ---- scores: tanh + wv-reduction, strided into PSUM.
            # Groups (4 q each) pack spb-to-a-bank side-by-side along the
            # free dim when vbe is small, cutting drain count. ----
            spb = min(2, max(1, 512 // vbe))  # segments (groups) per bank
            st_tiles = []
            gps = None
            for blk in range(QS // QBLK):  # blocks of QBLK q's
                S = []
                for hc in range(HC):
                    s_t = s_p.tile([128, QBLK, 512], BF16, tag=f"s{hc}")
                    nb_tot = 4 if vb < 128 else min(8, vb // 96)  # bias q's per (b, hc)
                    nb = (nb_tot + 1) // 2 if blk == 0 else nb_tot // 2
                    if bi == 0 and blk == 0:
                        nb = min(QBLK, nb + 4)
                    for qi in range(nb):
                        col = b * QS + blk * QBLK + qi
                        nc.scalar.activation(
                            s_t[:, qi, 0:vbe], kf[:, hc, 0:vbe], AFT.Tanh,
                            bias=qf[:, hc, col:col + 1])
                    for qi in range(nb, QBLK):
                        col = b * QS + blk * QBLK + qi
                        nc.vector.tensor_scalar_add(
                            s_t[:, qi, 0:vbe], kf[:, hc, 0:vbe],
                            qf[:, hc, col:col + 1])
                    # one big in-place tanh over the DVE-written range
                    nc.scalar.activation(
                        s_t[:, nb:QBLK, 0:vbe], s_t[:, nb:QBLK, 0:vbe],
                        AFT.Tanh)
                    S.append(s_t)
                for g2 in range(QBLK // 4):  # 4-q groups
                    g = blk * (QBLK // 4) + g2
                    seg = g % spb
                    if seg == 0:
                        gps = sc_ps.tile([128, 512], F32, tag="ps512")
                    lo = seg * vbe
                    for jj in range(4):
                        qi = g2 * 4 + jj
                        row = gps[32 * jj:32 * jj + 1, lo:lo + vbe]
                        if jj == 0:
                            # M=128 opener: initializes all 128 rows of seg
                            nc.tensor.matmul(
                                gps[:, lo:lo + vbe], wvp0[:],
                                S[0][:, qi, 0:vbe],
                                start=True, stop=False, tile_position=(0, 0),
                                skip_group_check=True)
                        else:
                            nc.tensor.matmul(
                                row, wvc[:, 0:1], S[0][:, qi, 0:vbe],
                                start=False, stop=False,
                                tile_position=(0, 32 * jj),
                                skip_group_check=True)
                        nc.tensor.matmul(
                            row, wvc[:, 1:2], S[1][:, qi, 0:vbe],
                            start=False, stop=(jj == 3),
                            tile_position=(0, 32 * jj),
                            skip_group_check=True)
                    if seg == spb - 1 or g == 7:
                        nseg = seg + 1
                        stt = st_p.tile([128, 512], F32, tag="st")
                        nc.vector.tensor_copy(
                            stt[:, 0:nseg * vbe], gps[:, 0:nseg * vbe])
                        st_tiles.append(stt)
                if blk == 0 and bi + 1 < B:
                    emit_kft(border[bi + 1])

            # ---- gather strided rows -> compact [32, vb] ----
            comp = sm_p.tile([32, 512], F32, tag="comp")
            for g in range(8):
                seg = g % spb
                nc.sync.dma_start(
                    comp[g * 4:(g + 1) * 4, 0:vb],
                    st_tiles[g // spb][0:128:32, seg * vbe:seg * vbe + vb])

            # ---- softmax (no max subtraction; denom via accum_out) ----
            ex = sm_p.tile([32, 512], BF16, tag="ex")
            den = sm_p.tile([32, 1], F32, tag="den")
            nc.scalar.activation(
                ex[:, 0:vb], comp[:, 0:vb], AFT.Exp, accum_out=den[:])
            rec = sm_p.tile([32, 1], F32, tag="rec")
            nc.vector.reciprocal(rec[:], den[:])

            # ---- attnT via PE transpose + final attn @ values ----
            ops = out_ps.tile([32, DV], F32, tag="outps")
            for kc in range(nkc):
                ksz = min(128, vb - kc * 128)
                tps = tr_ps.tile([128, 32], BF16, tag="trps")
                nc.tensor.transpose(
                    tps[0:ksz, :], ex[:, kc * 128:kc * 128 + ksz], eye[:])
                at = at_p.tile([128, 32], BF16, tag="at")
                nc.vector.tensor_copy(at[0:ksz, :], tps[0:ksz, :])
                vt, vksz = all_vts[b][kc]
                assert vksz == ksz
                nc.tensor.matmul(
                    ops[:], at[0:ksz, :], vt[0:ksz],
                    start=(kc == 0), stop=(kc == nkc - 1))
            ob = ob_p.tile([32, DV], F32, tag="ob")
            # fused 1/denom scale on the PSUM drain (per-partition scale);
            # last batch goes through DVE so ACT's tail is free
            if bi == B - 1:
                nc.vector.tensor_scalar_mul(ob[:], ops[:], rec[:])
            else:
                nc.scalar.mul(ob[:], ops[:], rec[:])
            nc.sync.dma_start(outs["out"][b], ob[:])


def _host_prep(queries, keys, values, Wq, Wk, wv):
    import ml_dtypes
    bf = ml_dtypes.bfloat16
    keysT = np.ascontiguousarray(keys.transpose(0, 2, 1)).astype(bf)
    values_bf = values.astype(bf)
    Wq_bf = Wq.astype(bf)
    Wk_bf = Wk.astype(bf)
    wv_col = np.ascontiguousarray(wv.reshape(HC, 128).T).astype(bf)  # [p, hc]
    wv_pad0 = np.zeros((128, 128), np.float32)
    wv_pad0[:, 0] = wv[0:128]
    wv_pad0 = wv_pad0.astype(bf)
    eye32 = np.eye(32, dtype=np.float32).astype(bf)
    qTs = []
    for c in range(N_CORES):
        qT = np.concatenate(
            [queries[b, c * QS:(c + 1) * QS, :].T for b in range(B)], axis=1)
        qTs.append(np.ascontiguousarray(qT).astype(bf))  # (512, 128)
    return keysT, values_bf, Wq_bf, Wk_bf, wv_col, wv_pad0, eye32, qTs


def _build_program(vbs):
    key = tuple(vbs)
    if key in _PROGRAM_CACHE:
        return _PROGRAM_CACHE[key]
    nc = bacc.Bacc("TRN2", target_bir_lowering=False, debug=False,
                   num_devices=N_CORES)
    ins = {
        "qT": nc.dram_tensor("qT", (DQ, 128), BF16, kind="ExternalInput").ap(),
        "keysT": nc.dram_tensor("keysT", (B, DK, LK), BF16,
                                kind="ExternalInput").ap(),
        "values": nc.dram_tensor("values", (B, LK, DV), BF16,
                                 kind="ExternalInput").ap(),
        "Wq": nc.dram_tensor("Wq", (DQ, H), BF16, kind="ExternalInput").ap(),
        "Wk": nc.dram_tensor("Wk", (DK, H), BF16, kind="ExternalInput").ap(),
        "wv_col": nc.dram_tensor("wv_col", (128, HC), BF16,
                                 kind="ExternalInput").ap(),
        "wv_pad0": nc.dram_tensor("wv_pad0", (128, 128), BF16,
                                  kind="ExternalInput").ap(),
        "eye32": nc.dram_tensor("eye32", (32, 32), BF16,
                                kind="ExternalInput").ap(),
    }
    outs = {
        "out": nc.dram_tensor("out", (B, QS, DV), F32,
                              kind="ExternalOutput").ap(),
    }
    with tile.TileContext(nc) as tc:
        build_body(tc, outs, ins, vbs)
    nc.compile()
    _PROGRAM_CACHE[key] = nc
    return nc


def kernel(queries, keys, values, valid_lens, Wq, Wk, wv):
    global last_results
    queries = np.asarray(queries, np.float32)
    keys = np.asarray(keys, np.float32)
    values = np.asarray(values, np.float32)
    Wq = np.asarray(Wq, np.float32)
    Wk = np.asarray(Wk, np.float32)
    wv = np.asarray(wv, np.float32)
    vbs = [int(min(max(int(v), 1), LK)) for v in np.asarray(valid_lens)]

    keysT, values_bf, Wq_bf, Wk_bf, wv_col, wv_pad0, eye32, qTs = _host_prep(
        queries, keys, values, Wq, Wk, wv)
    nc = _build_program(vbs)

    shared = {
        "keysT": keysT, "values": values_bf, "Wq": Wq_bf, "Wk": Wk_bf,
        "wv_col": wv_col, "wv_pad0": wv_pad0, "eye32": eye32,
    }
    in_maps = [dict(shared, qT=qTs[c]) for c in range(N_CORES)]
    res = bass_utils.run_bass_kernel_spmd(
        nc, in_maps, core_ids=list(range(N_CORES)),
        trace=bool(os.environ.get("BASSK_TRACE")),
    )
    last_results = res
    full = np.empty((B, LQ, DV), np.float32)
    for c in range(N_CORES):
        full[:, c * QS:(c + 1) * QS, :] = res.results[c]["out"]
    return full


# revision 22
# speedup vs baseline: 1.0151x; 1.0151x over previous
"""Additive attention (Bahdanau) Trainium2 Bass kernel.

Problem (hardcoded shapes):
  B=4, LQ=256, LK=512, DQ=DK=DV=512, H=256, f32.
  out[b,q,v] = sum_k softmax_k(score)[b,q,k] * values[b,k,v]
  score[b,q,k] = sum_h wv[h] * tanh( (queries@Wq)[b,q,h] + (keys@Wk)[b,k,h] )
  with softmax masked to k < valid_lens[b].

Sharding: 8 cores, each core handles ALL batches but a 32-row slice of LQ
(q-rows c*32:(c+1)*32 of every batch).  No collectives: host slices inputs
and concatenates outputs.

Device-side layout: h lives on SBUF partitions.
  qfT[h, q] and kfT[h, k] come straight out of PE matmuls (host pre-transposes
  queries and keys so the d-contraction is on partitions).  Per (q, h-chunk),
  DVE builds tanh-inputs kfT + qfT[:,q] in bf16 (per-partition scalar add,
  2x packed mode) -- a few per (b,hc) go through ACT's fused bias+tanh
  instead, to balance the two engines.  ACT does one big tanh per 16-q block,
  and PE reduces over h against wv with M=1 matmuls packed 4-to-a-PSUM-bank
  via 32-aligned col-tiling.  Softmax skips max-subtraction (|score| <=
  |wv|_1 ~ 13, exp can't overflow); the denominator comes free from the
  exp's accum_out.  Everything is trimmed to k < valid_lens[b] (host-known
  at build time), which skips the masked work entirely.
"""
import os
from contextlib import ExitStack

import numpy as np

import concourse.bacc as bacc
import concourse.bass as bass
import concourse.tile as tile
from concourse import mybir
from concourse import bass_utils

AFT = mybir.ActivationFunctionType
F32 = mybir.dt.float32
BF16 = mybir.dt.bfloat16

B, LQ, LK = 4, 256, 512
DQ, DK, DV = 512, 512, 512
H = 256
N_CORES = 8
QS = LQ // N_CORES  # 32 q rows per core
QBLK = 16           # q rows per tanh batch
N_BIAS = 3          # q rows per (b, hc, blk0) routed through ACT bias-tanh
N_GP = 0            # q rows per (b, hc, blk) routed through GPSIMD adds
HC = H // 128       # h chunks (2)
DC = DQ // 128      # d chunks (4)

_PROGRAM_CACHE: dict = {}
last_results = None  # for test harness introspection


def build_body(tc, outs, ins, vbs):
    """Emit the per-core program.  outs/ins are dicts of bass.AP (DRAM)."""
    nc = tc.nc
    ctx = ExitStack()
    with ctx:
        consts = ctx.enter_context(tc.tile_pool(name="consts", bufs=1))
        kin_p = ctx.enter_context(tc.tile_pool(name="kin", bufs=3))
        kf_p = ctx.enter_context(tc.tile_pool(name="kf", bufs=4))
        s_p = ctx.enter_context(tc.tile_pool(name="s", bufs=3))
        st_p = ctx.enter_context(tc.tile_pool(name="st", bufs=12))
        sm_p = ctx.enter_context(tc.tile_pool(name="sm", bufs=3))
        at_p = ctx.enter_context(tc.tile_pool(name="at", bufs=4))
        val_p = ctx.enter_context(tc.tile_pool(name="val", bufs=8))
        ob_p = ctx.enter_context(tc.tile_pool(name="ob", bufs=2))
        # PSUM pools: 4+2+2 = 8 banks exactly (projections and score tiles
        # share one 4-deep pool)
        main_ps = ctx.enter_context(tc.tile_pool(name="mainps", bufs=4, space="PSUM"))
        proj_ps = main_ps
        sc_ps = main_ps
        tr_ps = ctx.enter_context(tc.tile_pool(name="trps", bufs=2, space="PSUM"))
        out_ps = ctx.enter_context(tc.tile_pool(name="outps", bufs=2, space="PSUM"))

        # ---- constants (kfT inputs first: they gate the critical path) ----
        wk_t = consts.tile([128, DC, H], BF16)
        nc.scalar.dma_start(wk_t[:], ins["Wk"].rearrange("(c p) h -> p c h", p=128))
        wq_t = consts.tile([128, DC, H], BF16)
        nc.scalar.dma_start(wq_t[:], ins["Wq"].rearrange("(c p) h -> p c h", p=128))
        qt_t = consts.tile([128, DC, 128], BF16)
        nc.scalar.dma_start(qt_t[:], ins["qT"].rearrange("(c p) q -> p c q", p=128))
        wvc = consts.tile([128, HC], BF16)
        nc.scalar.dma_start(wvc[:], ins["wv_col"][:])
        wvp0 = consts.tile([128, 128], BF16)
        nc.scalar.dma_start(wvp0[:], ins["wv_pad0"][:])
        eye = consts.tile([32, 32], BF16)
        nc.scalar.dma_start(eye[:], ins["eye32"][:])

        # ---- kfT projections upfront, largest batch first; qfT right after
        # the first so the first adds are unblocked early ----
        border = sorted(range(B), key=lambda b: -vbs[b])
        kfs = {}
        qf = None
        for bi, b in enumerate(border):
            vb = vbs[b]
            vbe = vb + (vb & 1)
            kt = kin_p.tile([128, DC, 512], BF16, tag="kin")
            nc.sync.dma_start(
                kt[:, :, 0:vbe],
                ins["keysT"][b].rearrange("(c p) k -> p c k", p=128)[:, :, 0:vbe])
            kf = kf_p.tile([128, HC, 512], BF16, tag="kf")
            for hc in range(HC):
                kps = proj_ps.tile([128, 512], F32, tag="ps512")
                for dc in range(DC):
                    nc.tensor.matmul(
                        kps[:, 0:vbe],
                        wk_t[:, dc, hc * 128:(hc + 1) * 128],
                        kt[:, dc, 0:vbe],
                        start=(dc == 0), stop=(dc == DC - 1),
                    )
                nc.vector.tensor_copy(kf[:, hc, 0:vbe], kps[:, 0:vbe])
            kfs[b] = kf
            if bi == 0:
                # ---- qfT[h, 4b*32q] ----
                qf = consts.tile([128, HC, 128], F32)
                for hc in range(HC):
                    qps = proj_ps.tile([128, 512], F32, tag="ps512")
                    for dc in range(DC):
                        nc.tensor.matmul(
                            qps[:, 0:128],
                            wq_t[:, dc, hc * 128:(hc + 1) * 128],
                            qt_t[:, dc, :],
                            start=(dc == 0), stop=(dc == DC - 1),
                        )
                    nc.vector.tensor_copy(qf[:, hc, :], qps[:, 0:128])

        # ---- values stream (bf16) ----
        all_vts = {}
        for b in border:
            vb = vbs[b]
            nkc_b = (vb + 127) // 128
            nfull = vb // 128
            vt = val_p.tile([128, nkc_b, DV], BF16, tag=f"val{nkc_b}")
            if nfull:
                nc.sync.dma_start(
                    vt[:, 0:nfull, :],
                    ins["values"][b, 0:nfull * 128, :].rearrange(
                        "(c p) v -> p c v", p=128))
            if vb % 128:
                nc.sync.dma_start(
                    vt[0:vb % 128, nfull, :],
                    ins["values"][b, nfull * 128:vb, :])
            all_vts[b] = [(vt[:, kc, :], min(128, vb - kc * 128))
                          for kc in range(nkc_b)]

        for bi, b in enumerate(border):
            vb = vbs[b]
            vbe = vb + (vb & 1)  # even pad so DVE tensor_scalar packs 2/cycle
            nkc = (vb + 127) // 128
            kf = kfs[b]

            # ---- scores: tanh + wv-reduction, strided into PSUM.
            # Groups (4 q each) pack spb-to-a-bank side-by-side along the
            # free dim when vbe is small, cutting drain count. ----
            spb = min(2, max(1, 512 // vbe))  # segments (groups) per bank
            st_tiles = []
            gps = None
            for blk in range(QS // QBLK):  # blocks of QBLK q's
                S = []
                for hc in range(HC):
                    s_t = s_p.tile([128, QBLK, 512], BF16, tag=f"s{hc}")
                    nb_tot = 4 if vb < 128 else min(8, vb // 96)  # bias q's per (b, hc)
                    nb = (nb_tot + 1) // 2 if blk == 0 else nb_tot // 2
                    if bi == 0 and blk == 0:
                        nb = min(QBLK, nb + 4)
                    for qi in range(nb):
                        col = b * QS + blk * QBLK + qi
                        nc.scalar.activation(
                            s_t[:, qi, 0:vbe], kf[:, hc, 0:vbe], AFT.Tanh,
                            bias=qf[:, hc, col:col + 1])
                    for qi in range(nb, QBLK):
                        col = b * QS + blk * QBLK + qi
                        nc.vector.tensor_scalar_add(
                            s_t[:, qi, 0:vbe], kf[:, hc, 0:vbe],
                            qf[:, hc, col:col + 1])
                    # one big in-place tanh over the DVE-written range
                    nc.scalar.activation(
                        s_t[:, nb:QBLK, 0:vbe], s_t[:, nb:QBLK, 0:vbe],
                        AFT.Tanh)
                    S.append(s_t)
                for g2 in range(QBLK // 4):  # 4-q groups
                    g = blk * (QBLK // 4) + g2
                    seg = g % spb
                    if seg == 0:
                        gps = sc_ps.tile([128, 512], F32, tag="ps512")
                    lo = seg * vbe
                    for jj in range(4):
                        qi = g2 * 4 + jj
                        row = gps[32 * jj:32 * jj + 1, lo:lo + vbe]
                        if jj == 0:
                            # M=128 opener: initializes all 128 rows of seg
                            nc.tensor.matmul(
                                gps[:, lo:lo + vbe], wvp0[:],
                                S[0][:, qi, 0:vbe],
                                start=True, stop=False, tile_position=(0, 0),
                                skip_group_check=True)
                        else:
                            nc.tensor.matmul(
                                row, wvc[:, 0:1], S[0][:, qi, 0:vbe],
                                start=False, stop=False,
                                tile_position=(0, 32 * jj),
                                skip_group_check=True)
                        nc.tensor.matmul(
                            row, wvc[:, 1:2], S[1][:, qi, 0:vbe],
                            start=False, stop=(jj == 3),
                            tile_position=(0, 32 * jj),
                            skip_group_check=True)
                    if seg == spb - 1 or g == 7:
                        nseg = seg + 1
                        stt = st_p.tile([128, 512], F32, tag="st")
                        nc.vector.tensor_copy(
                            stt[:, 0:nseg * vbe], gps[:, 0:nseg * vbe])
                        st_tiles.append(stt)

            # ---- gather strided rows -> compact [32, vb] ----
            comp = sm_p.tile([32, 512], F32, tag="comp")
            for g in range(8):
                seg = g % spb
                nc.sync.dma_start(
                    comp[g * 4:(g + 1) * 4, 0:vb],
                    st_tiles[g // spb][0:128:32, seg * vbe:seg * vbe + vb])

            # ---- softmax (no max subtraction; denom via accum_out) ----
            ex = sm_p.tile([32, 512], BF16, tag="ex")
            den = sm_p.tile([32, 1], F32, tag="den")
            nc.scalar.activation(
                ex[:, 0:vb], comp[:, 0:vb], AFT.Exp, accum_out=den[:])
            rec = sm_p.tile([32, 1], F32, tag="rec")
            nc.vector.reciprocal(rec[:], den[:])

            # ---- attnT via PE transpose + final attn @ values ----
            ops = out_ps.tile([32, DV], F32, tag="outps")
            for kc in range(nkc):
                ksz = min(128, vb - kc * 128)
                tps = tr_ps.tile([128, 32], BF16, tag="trps")
                nc.tensor.transpose(
                    tps[0:ksz, :], ex[:, kc * 128:kc * 128 + ksz], eye[:])
                at = at_p.tile([128, 32], BF16, tag="at")
                nc.vector.tensor_copy(at[0:ksz, :], tps[0:ksz, :])
                vt, vksz = all_vts[b][kc]
                assert vksz == ksz
                nc.tensor.matmul(
                    ops[:], at[0:ksz, :], vt[0:ksz],
                    start=(kc == 0), stop=(kc == nkc - 1))
            ob = ob_p.tile([32, DV], F32, tag="ob")
            # fused 1/denom scale on the PSUM drain (per-partition scale);
            # last batch goes through DVE so ACT's tail is free
            if bi == B - 1:
                nc.vector.tensor_scalar_mul(ob[:], ops[:], rec[:])
            else:
                nc.scalar.mul(ob[:], ops[:], rec[:])
            nc.sync.dma_start(outs["out"][b], ob[:])


def _host_prep(queries, keys, values, Wq, Wk, wv):
    import ml_dtypes
    bf = ml_dtypes.bfloat16
    keysT = np.ascontiguousarray(keys.transpose(0, 2, 1)).astype(bf)
    values_bf = values.astype(bf)
    Wq_bf = Wq.astype(bf)
    Wk_bf = Wk.astype(bf)
    wv_col = np.ascontiguousarray(wv.reshape(HC, 128).T).astype(bf)  # [p, hc]
    wv_pad0 = np.zeros((128, 128), np.float32)
    wv_pad0[:, 0] = wv[0:128]
    wv_pad0 = wv_pad0.astype(bf)
    eye32 = np.eye(32, dtype=np.float32).astype(bf)
    qTs = []
    for c in range(N_CORES):
        qT = np.concatenate(
            [queries[b, c * QS:(c + 1) * QS, :].T for b in range(B)], axis=1)
        qTs.append(np.ascontiguousarray(qT).astype(bf))  # (512, 128)
    return keysT, values_bf, Wq_bf, Wk_bf, wv_col, wv_pad0, eye32, qTs


def _build_program(vbs):
    key = tuple(vbs)
    if key in _PROGRAM_CACHE:
        return _PROGRAM_CACHE[key]
    nc = bacc.Bacc("TRN2", target_bir_lowering=False, debug=False,
                   num_devices=N_CORES)
    ins = {
        "qT": nc.dram_tensor("qT", (DQ, 128), BF16, kind="ExternalInput").ap(),
        "keysT": nc.dram_tensor("keysT", (B, DK, LK), BF16,
                                kind="ExternalInput").ap(),
        "values": nc.dram_tensor("values", (B, LK, DV), BF16,
                                 kind="ExternalInput").ap(),
        "Wq": nc.dram_tensor("Wq", (DQ, H), BF16, kind="ExternalInput").ap(),
        "Wk": nc.dram_tensor("Wk", (DK, H), BF16, kind="ExternalInput").ap(),
        "wv_col": nc.dram_tensor("wv_col", (128, HC), BF16,
                                 kind="ExternalInput").ap(),
        "wv_pad0": nc.dram_tensor("wv_pad0", (128, 128), BF16,
                                  kind="ExternalInput").ap(),
        "eye32": nc.dram_tensor("eye32", (32, 32), BF16,
                                kind="ExternalInput").ap(),
    }
    outs = {
        "out": nc.dram_tensor("out", (B, QS, DV), F32,
                              kind="ExternalOutput").ap(),
    }
    with tile.TileContext(nc) as tc:
        build_body(tc, outs, ins, vbs)
    nc.compile()
    _PROGRAM_CACHE[key] = nc
    return nc


def kernel(queries, keys, values, valid_lens, Wq, Wk, wv):
    global last_results
    queries = np.asarray(queries, np.float32)
    keys = np.asarray(keys, np.float32)
    values = np.asarray(values, np.float32)
    Wq = np.asarray(Wq, np.float32)
    Wk = np.asarray(Wk, np.float32)
    wv = np.asarray(wv, np.float32)
    vbs = [int(min(max(int(v), 1), LK)) for v in np.asarray(valid_lens)]

    keysT, values_bf, Wq_bf, Wk_bf, wv_col, wv_pad0, eye32, qTs = _host_prep(
        queries, keys, values, Wq, Wk, wv)
    nc = _build_program(vbs)

    shared = {
        "keysT": keysT, "values": values_bf, "Wq": Wq_bf, "Wk": Wk_bf,
        "wv_col": wv_col, "wv_pad0": wv_pad0, "eye32": eye32,
    }
    in_maps = [dict(shared, qT=qTs[c]) for c in range(N_CORES)]
    res = bass_utils.run_bass_kernel_spmd(
        nc, in_maps, core_ids=list(range(N_CORES)),
        trace=bool(os.environ.get("BASSK_TRACE")),
    )
    last_results = res
    full = np.empty((B, LQ, DV), np.float32)
    for c in range(N_CORES):
        full[:, c * QS:(c + 1) * QS, :] = res.results[c]["out"]
    return full
